# revision 5
# baseline (speedup 1.0000x reference)
"""Trainium2 Bass kernel for OctahedralCavityProcessor.

Sharding: data-parallel over batch (B=8 -> 8 cores, zero collectives).
Per core (one batch element), in x/s units where s = max|x|*1.01/127
(the uint8 output scale; host pre-divides x, folds s into W1 and the
attention output projection, and dequantizes the output):

  phase A: cavity pooling featT[c,k] = sum_p xT[p,c] * mask[p,k]
           - x streamed once in fp16 (24.5 MB), PE-transposed in groups
             of 8 chunks (one PSUM bank per group), PSUM->SBUF copies
             alternate DVE/Act
           - pooling matmul flipped: xt chunk is the stationary operand,
             the K=14 fp8 mask columns are the moving operand
           - 1/count (and s) folded into W1 host-side; mask is 0/1 fp8
           - the last NKEEP=35 x blocks stay SBUF-resident for phase C
  phase B: per-cavity MLP (fp8 weights, batched over all 14 cavities,
           biases via DVE adds) + 14-token MHA (fp32 on-chip); output
           projection emits att/s + 128 as [K, C] directly
  phase C: out[c,p] = x[c,p] + att[c, nearest[p]], processed cached
           blocks first:
           - att/s @ onehot(fp8) into PSUM; +x via PE identity-matmul
             accumulate (Act-converted blocks) or via the DVE add itself
           - stored as uint8 (round(out/s)+128; the +128 bias rides the
             onehot matmul since each point has exactly one 1), Pool
             engine runs the store DMAs
  HBM IO per core ~48 MB: x fp16 once + ~7 uncached re-reads, fp8
  mask/onehot, fp8 MLP weights, uint8 out.  Error budget: dominated by
  fp16 x rounding + 0.5-step uint8 rounding ~= 4.5e-3 of output scale.
"""
import numpy as np

import concourse.bass as bass
import concourse.tile as tile
from concourse import mybir
from concourse.bass_utils import run_bass_kernel_spmd
from concourse.vector_clock import ScopedClock, VectorClock
from contextlib import ExitStack

F32 = mybir.dt.float32
F16 = mybir.dt.float16
F8 = mybir.dt.float8e4
U8 = mybir.dt.uint8

B, C, P, K, H = 8, 128, 100000, 14, 8
C2 = 2 * C
Dh = C // H
RADIUS = np.float32(0.5)

CHA = 128                     # point-chunk (transpose width)
CB = 2048                     # block: 16 chunks; also phase C block
NB = (P + CB - 1) // CB       # 49 blocks
P2 = NB * CB                  # 100352: x/onehot zero-padded to full blocks
NA4 = P2 // CHA               # 784 chunks, all full
NKEEP = 35                    # x blocks kept in SBUF for phase C


def _legalize_bir_waits(bir_json: bytes) -> bytes:
    """walrus here accepts at most ONE sync-wait command per instruction.
    Tile's scheduler may attach several.  Hoist the extras onto NoOp
    instructions inserted immediately before, on the same engine (the
    engine executes serially, so waiting one-at-a-time is equivalent)."""
    import json as _json

    d = _json.loads(bir_json)
    changed = False
    for fn in d.get("functions", []):
        for blk in fn.get("blocks", []):
            insts = blk.get("instructions", [])
            out = []
            for ins in insts:
                waits = (ins.get("sync_info") or {}).get("on_wait", [])
                if len(waits) > 1:
                    changed = True
                    for i, w in enumerate(waits[:-1]):
                        out.append({
                            "debug": ins.get("debug", 0),
                            "engine": ins["engine"],
                            "ins": [],
                            "name": f"{ins['name']}-wsplit{i}",
                            "opcode": "NoOp",
                            "outs": [],
                            "sync_info": {"on_update": [], "on_wait": [w]},
                            "text_hint": "wait_split",
                        })
                    ins["sync_info"]["on_wait"] = [waits[-1]]
                out.append(ins)
            blk["instructions"] = out
    if not changed:
        return bir_json
    return _json.dumps(d).encode()


def _install_wait_legalizer():
    import concourse.bass2jax as _b2j

    orig = _b2j.compile_bir_kernel
    if getattr(orig, "_wait_legalized", False):
        return

    def patched(bir_json, tmpdir, neff_name="file.neff"):
        return orig(_legalize_bir_waits(bir_json), tmpdir, neff_name=neff_name)

    patched._wait_legalized = True
    _b2j.compile_bir_kernel = patched


_install_wait_legalizer()


class SplitDrainTileContext(tile.TileContext):
    """The walrus build here only accepts ONE sync-wait command per
    instruction; stock TileContext puts every live sem wait on the tail
    Drain.  Split them across nop instructions instead."""

    def _drain_and_barrier(self, tick_clock, wait_clock):
        gc = tick_clock.global_clock
        n = len(gc)
        for i in range(n):
            if gc[i] <= 0:
                continue
            vec = [gc[j] if j == i else 0 for j in range(n)]
            nop = self.nc.sync.nop(nofuse=True, hint="tail_drain_split")
            wait_clock.add_sem_waits(nop.ins, ScopedClock({None: VectorClock(vec)}))
        self.nc.sync.drain()
        self.nc.all_engine_barrier()
        assert self.sems is not None
        popped = self.nc._tile_sem_poison_stack.pop()
        assert popped is self._sem_poison
        self.nc.clear_and_free_semaphores(list(self.sems.allocated().values()))
        self.nc.all_engine_barrier()


def build_program(reps=1):
    nc = bass.Bass()

    x_d = nc.dram_tensor("x", [C, P2], F16, kind="ExternalInput")
    # maskA[p, c*K+k] = 0/1 mask of point c*CHA+p, cavity k (fp16)
    maskA_d = nc.dram_tensor("maskA", [CHA, NA4 * K], F8, kind="ExternalInput")
    onehot_d = nc.dram_tensor("onehot", [K, P2], F8, kind="ExternalInput")
    # w1c[c, k*C2+d] = w1[k,d,c] / count_k   (inv-count folded in)
    w1c_d = nc.dram_tensor("w1c", [C, K * C2], F8, kind="ExternalInput")
    # w2c[dd, (k*2+j)*C + c] = w2[k, c, j*C+dd]
    w2c_d = nc.dram_tensor("w2c", [C, 2 * K * C], F8, kind="ExternalInput")
    b1t_d = nc.dram_tensor("b1t", [C, 2 * K], F32, kind="ExternalInput")
    b2t_d = nc.dram_tensor("b2t", [C, K], F32, kind="ExternalInput")
    wqkv_d = nc.dram_tensor("wqkv", [C, 3 * C], F8, kind="ExternalInput")
    wo_d = nc.dram_tensor("wo", [Dh, H * C], F32, kind="ExternalInput")
    qb_d = nc.dram_tensor("qb", [Dh, H * K], F32, kind="ExternalInput")
    kb_d = nc.dram_tensor("kb", [Dh, H * K], F32, kind="ExternalInput")
    vb_d = nc.dram_tensor("vb", [K, C], F32, kind="ExternalInput")
    obr_d = nc.dram_tensor("obr", [K, C], F32, kind="ExternalInput")
    zc_d = nc.dram_tensor("zc", [C, 1], F32, kind="ExternalInput")
    identT_d = nc.dram_tensor("identT", [CHA, CHA], F16, kind="ExternalInput")
    ident14_d = nc.dram_tensor("ident14", [K, K], F32, kind="ExternalInput")
    out_d = nc.dram_tensor("out", [C, P], U8, kind="ExternalOutput")

    Id = mybir.ActivationFunctionType.Identity

    with SplitDrainTileContext(nc) as tc:
      for _rep in range(reps):
        with ExitStack() as octx:
            cpool = octx.enter_context(tc.tile_pool(name="consts", bufs=1))
            cache_pool = octx.enter_context(
                tc.tile_pool(name="xcache", bufs=NKEEP))


            # identity + mask first on the Act queue (phase A pooling
            # head-of-line depends on them); weights after (needed at B)
            pf_pool = octx.enter_context(tc.tile_pool(name="pf", bufs=3))
            identT_s = cpool.tile([CHA, CHA], F16, tag="identT")
            nc.scalar.dma_start(identT_s[:], identT_d[:])
            m_free_ctx = ExitStack()
            m_pool = m_free_ctx.enter_context(tc.tile_pool(name="mA", bufs=1))
            feat_pool = m_free_ctx.enter_context(
                tc.tile_pool(name="feat_ps", bufs=1, space="PSUM"))
            m_all = m_pool.tile([CHA, NA4 * K], F8, tag="m")
            MQ = NA4 * K // 4
            for q in range(4):
                nc.scalar.dma_start(m_all[:, q * MQ:(q + 1) * MQ],
                                    maskA_d[:, q * MQ:(q + 1) * MQ])
            ident14_s = cpool.tile([K, K], F32, tag="ident14")
            nc.scalar.dma_start(ident14_s[:], ident14_d[:])
            w1_s = cpool.tile([C, K * C2], F8, tag="w1")
            nc.scalar.dma_start(w1_s[:], w1c_d[:])
            w2_s = cpool.tile([C, 2 * K * C], F8, tag="w2")
            nc.scalar.dma_start(w2_s[:], w2c_d[:])
            b1t_s = cpool.tile([C, 2 * K], F32, tag="b1t")
            nc.scalar.dma_start(b1t_s[:], b1t_d[:])
            b2t_s = cpool.tile([C, K], F32, tag="b2t")
            nc.scalar.dma_start(b2t_s[:], b2t_d[:])
            wqkv_s = cpool.tile([C, 3 * C], F8, tag="wqkv")
            nc.scalar.dma_start(wqkv_s[:], wqkv_d[:])
            wo_s = cpool.tile([Dh, H * C], F32, tag="wo")
            nc.scalar.dma_start(wo_s[:], wo_d[:])
            qb_s = cpool.tile([Dh, H * K], F32, tag="qb")
            nc.scalar.dma_start(qb_s[:], qb_d[:])
            kb_s = cpool.tile([Dh, H * K], F32, tag="kb")
            nc.scalar.dma_start(kb_s[:], kb_d[:])
            vb_s = cpool.tile([K, C], F32, tag="vb")
            nc.scalar.dma_start(vb_s[:], vb_d[:])
            obr_s = cpool.tile([K, C], F32, tag="obr")
            nc.scalar.dma_start(obr_s[:], obr_d[:])
            zc_s = cpool.tile([C, 1], F32, tag="zc")
            nc.scalar.dma_start(zc_s[:], zc_d[:])

            featT_ps = feat_pool.tile([C, K], F32, tag="featT_ps")
            cache_tiles = {}

            # ---------------- phase A: cavity pooling ----------------
            with ExitStack() as actx:
                xg_pool = actx.enter_context(tc.tile_pool(name="xg", bufs=3))
                xt_pool = actx.enter_context(tc.tile_pool(name="xt", bufs=4))
                tp_pool = actx.enter_context(
                    tc.tile_pool(name="tp", bufs=4, space="PSUM"))

                grp = 0
                for g in range(NB):
                    g0 = g * CB
                    pool = cache_pool if g >= NB - NKEEP else xg_pool
                    xg_t = pool.tile([C, CB], F16, tag="xg")
                    nc.sync.dma_start(xg_t[:], x_d[:, g0:g0 + CB])
                    if g >= NB - NKEEP:
                        cache_tiles[g] = xg_t
                    for s in (0, 8):
                        tp_t = tp_pool.tile([CHA, 8 * C], F16, tag="tp")
                        xt_s = xt_pool.tile([CHA, 8 * C], F16, tag="xts")
                        for i in range(8):
                            off = (s + i) * CHA
                            nc.tensor.matmul(
                                tp_t[:, i * C:(i + 1) * C],
                                lhsT=xg_t[:, off:off + CHA],
                                rhs=identT_s[:],
                                is_transpose=True,
                                start=(i == 0),
                                stop=(i == 7),
                            )
                        # GPSIMD cannot access PSUM on HW: DVE/Act only
                        if grp % 3 < 2:
                            nc.vector.tensor_copy(xt_s[:], tp_t[:])
                        else:
                            nc.scalar.activation(xt_s[:], tp_t[:], Id)
                        grp += 1
                        for i in range(8):
                            cg = g * 16 + s + i
                            nc.tensor.matmul(
                                featT_ps[:],
                                lhsT=xt_s[:, i * C:(i + 1) * C],
                                rhs=m_all[:, cg * K:(cg + 1) * K],
                                start=(cg == 0),
                                stop=(cg == NA4 - 1),
                            )

                # prefetch the first phase-C x blocks (their SBUF does not
                # overlap A pools, so these loads overlap phase B)
                for jb in range(3):
                    if jb not in cache_tiles:
                        t = pf_pool.tile([C, CB], F16, tag="pf")
                        nc.sync.dma_start(t[:], x_d[:, jb * CB:(jb + 1) * CB])
                        cache_tiles[jb] = t

            # ---------------- phase B: MLP + attention (fp32) ----------------
            ak_s = cpool.tile([K, C], F16, tag="ak")
            with ExitStack() as bctx:
                sp = bctx.enter_context(
                    tc.tile_pool(name="sp_ps", bufs=4, space="PSUM"))
                hp = bctx.enter_context(tc.tile_pool(name="hp", bufs=2))

                featT = cpool.tile([C, K], F16, tag="featT")
                nc.vector.tensor_copy(featT[:], featT_ps[:])
                procT = cpool.tile([C, K], F16, tag="procT")

                # batched MLP: all 2K hidden columns in one PSUM tile,
                # biases via DVE tensor_add (also moves PSUM->SBUF), one
                # relu / tanh pass instead of per-cavity activations
                ph = sp.tile([C, 2 * K], F32, tag="sps")
                for k in range(K):
                    nc.tensor.matmul(ph[:, 2 * k:2 * k + 1],
                                     lhsT=w1_s[:, k * C2:k * C2 + C],
                                     rhs=featT[:, k:k + 1],
                                     start=(k == 0), stop=False)
                    nc.tensor.matmul(ph[:, 2 * k + 1:2 * k + 2],
                                     lhsT=w1_s[:, k * C2 + C:(k + 1) * C2],
                                     rhs=featT[:, k:k + 1],
                                     start=False, stop=(k == K - 1))
                hb = hp.tile([C, 2 * K], F32, tag="hb")
                nc.vector.tensor_add(hb[:], ph[:], b1t_s[:])
                h_all = hp.tile([C, 2 * K], F16, tag="h")
                nc.vector.tensor_relu(h_all[:], hb[:])
                pp = sp.tile([C, K], F32, tag="sps")
                for k in range(K):
                    nc.tensor.matmul(pp[:, k:k + 1],
                                     lhsT=w2_s[:, (2 * k) * C:(2 * k + 1) * C],
                                     rhs=h_all[:, 2 * k:2 * k + 1],
                                     start=(k == 0), stop=False)
                    nc.tensor.matmul(pp[:, k:k + 1],
                                     lhsT=w2_s[:, (2 * k + 1) * C:(2 * k + 2) * C],
                                     rhs=h_all[:, 2 * k + 1:2 * k + 2],
                                     start=False, stop=(k == K - 1))
                ppb = hp.tile([C, K], F32, tag="ppb")
                nc.vector.tensor_add(ppb[:], pp[:], b2t_s[:])
                nc.scalar.activation(procT[:], ppb[:],
                                     mybir.ActivationFunctionType.Tanh,
                                     bias=zc_s[:])

                # ---- attention over K=14 cavities ----
                # q/k head-blocked [Dh, H*K]
                pq = sp.tile([Dh, H * K], F32, tag="sps")
                for h in range(H):
                    nc.tensor.matmul(pq[:, h * K:(h + 1) * K],
                                     lhsT=wqkv_s[:, h * Dh:(h + 1) * Dh],
                                     rhs=procT[:],
                                     start=(h == 0), stop=(h == H - 1))
                qh_s = cpool.tile([Dh, H * K], F32, tag="qT")
                nc.vector.tensor_add(qh_s[:], pq[:], qb_s[:])

                pk = sp.tile([Dh, H * K], F32, tag="sps")
                for h in range(H):
                    nc.tensor.matmul(pk[:, h * K:(h + 1) * K],
                                     lhsT=wqkv_s[:, C + h * Dh:C + (h + 1) * Dh],
                                     rhs=procT[:],
                                     start=(h == 0), stop=(h == H - 1))
                kh_s = cpool.tile([Dh, H * K], F32, tag="kT")
                nc.vector.tensor_add(kh_s[:], pk[:], kb_s[:])

                pv = sp.tile([K, C], F32, tag="sps")
                nc.tensor.matmul(pv[:], lhsT=procT[:], rhs=wqkv_s[:, 2 * C:3 * C])
                v_s = cpool.tile([K, C], F32, tag="v")
                nc.vector.tensor_add(v_s[:], pv[:], vb_s[:])

                psc = sp.tile([K, H * K], F32, tag="sps")
                for h in range(H):
                    nc.tensor.matmul(
                        psc[:, h * K:(h + 1) * K],
                        lhsT=qh_s[:, h * K:(h + 1) * K],
                        rhs=kh_s[:, h * K:(h + 1) * K],
                        start=(h == 0),
                        stop=(h == H - 1),
                    )
                negmax = cpool.tile([K, H], F32, tag="negmax")
                nc.vector.tensor_reduce(
                    out=negmax[:],
                    in_=psc[:].rearrange("p (h j) -> p h j", j=K),
                    op=mybir.AluOpType.max,
                    axis=mybir.AxisListType.X,
                    negate=True,
                )
                esc = cpool.tile([K, H * K], F32, tag="esc")
                for h in range(H):
                    nc.scalar.activation(
                        esc[:, h * K:(h + 1) * K], psc[:, h * K:(h + 1) * K],
                        mybir.ActivationFunctionType.Exp,
                        bias=negmax[:, h:h + 1],
                    )
                ssum = cpool.tile([K, H], F32, tag="ssum")
                nc.vector.tensor_reduce(
                    out=ssum[:],
                    in_=esc[:].rearrange("p (h j) -> p h j", j=K),
                    op=mybir.AluOpType.add,
                    axis=mybir.AxisListType.X,
                )
                rinv = cpool.tile([K, H], F32, tag="rinv")
                nc.vector.reciprocal(rinv[:], ssum[:])
                for h in range(H):
                    nc.vector.tensor_scalar_mul(
                        esc[:, h * K:(h + 1) * K], esc[:, h * K:(h + 1) * K],
                        rinv[:, h:h + 1],
                    )

                pat = sp.tile([K, H * K], F32, tag="sps")
                for h in range(H):
                    nc.tensor.matmul(
                        pat[:, h * K:(h + 1) * K],
                        lhsT=esc[:, h * K:(h + 1) * K],
                        rhs=ident14_s[:],
                        is_transpose=True,
                        start=(h == 0),
                        stop=(h == H - 1),
                    )
                at_s = cpool.tile([K, H * K], F32, tag="at")
                nc.vector.tensor_copy(at_s[:], pat[:])

                po = sp.tile([Dh, H * K], F32, tag="sps")
                for h in range(H):
                    nc.tensor.matmul(
                        po[:, h * K:(h + 1) * K],
                        lhsT=v_s[:, h * Dh:(h + 1) * Dh],
                        rhs=at_s[:, h * K:(h + 1) * K],
                        start=(h == 0),
                        stop=(h == H - 1),
                    )
                o_s = cpool.tile([Dh, H * K], F32, tag="o")
                nc.vector.tensor_copy(o_s[:], po[:])

                # out-projection directly as [K, C]: accumulate over heads
                pak = sp.tile([K, C], F32, tag="sps")
                for h in range(H):
                    nc.tensor.matmul(pak[:],
                                     lhsT=o_s[:, h * K:(h + 1) * K],
                                     rhs=wo_s[:, h * C:(h + 1) * C],
                                     start=(h == 0), stop=(h == H - 1))
                nc.vector.tensor_add(ak_s[:], pak[:], obr_s[:])

            m_free_ctx.close()

            # ---------------- phase C: gather-add ----------------
            with ExitStack() as cctx:
                xc_pool = cctx.enter_context(tc.tile_pool(name="xc", bufs=3))
                oh_pool = cctx.enter_context(tc.tile_pool(name="oh", bufs=3))
                oc_pool = cctx.enter_context(tc.tile_pool(name="oc", bufs=3))
                pc_pool = cctx.enter_context(
                    tc.tile_pool(name="pc", bufs=3, space="PSUM"))
                cpy = 0
                # interleave load-requiring blocks among cached ones so
                # the x-load stream runs ahead of compute instead of
                # starving it at the tail
                cached = [jb for jb in range(NB) if jb in cache_tiles]
                loaded = [jb for jb in range(NB) if jb not in cache_tiles]
                order = []
                acc = 0.0
                ratio = len(loaded) / max(len(cached), 1)
                li = 0
                for cb_ in cached:
                    order.append(cb_)
                    acc += ratio
                    while acc >= 1.0 and li < len(loaded):
                        order.append(loaded[li])
                        li += 1
                        acc -= 1.0
                order.extend(loaded[li:])
                for jb in order:
                    b0 = jb * CB
                    bw = min(CB, P - b0)
                    if jb in cache_tiles:
                        xc_t = cache_tiles[jb]
                    else:
                        xc_t = xc_pool.tile([C, CB], F16, tag="xc")
                        nc.sync.dma_start(xc_t[:], x_d[:, b0:b0 + CB])
                    oh_t = oh_pool.tile([K, CB], F8, tag="oh")
                    nc.sync.dma_start(oh_t[:], onehot_d[:, b0:b0 + CB])
                    oc_t = oc_pool.tile([C, CB], U8, tag="ocd")
                    for j in range(0, CB, 1024):
                        pc_t = pc_pool.tile([C, 1024], F32, tag="pc")
                        # ~half the pairs: DVE adds +x (and uint8 convert)
                        # straight from PSUM; rest: PE accumulates
                        # identity@x and Act converts.
                        dve_add = (cpy % 2 == 0)
                        for jj in (0, 512):
                            nc.tensor.matmul(pc_t[:, jj:jj + 512],
                                             lhsT=ak_s[:],
                                             rhs=oh_t[:, j + jj:j + jj + 512],
                                             start=True, stop=dve_add)
                            if not dve_add:
                                nc.tensor.matmul(pc_t[:, jj:jj + 512],
                                                 lhsT=identT_s[:],
                                                 rhs=xc_t[:, j + jj:j + jj + 512],
                                                 start=False, stop=True)
                        if dve_add:
                            nc.vector.tensor_add(oc_t[:, j:j + 1024], pc_t[:],
                                                 xc_t[:, j:j + 1024])
                        else:
                            nc.scalar.activation(oc_t[:, j:j + 1024], pc_t[:], Id)
                        cpy += 1
                    nc.gpsimd.dma_start(out_d[:, b0:b0 + bw], oc_t[:, :bw])

    return nc


def prep_host(points, cavities, w1, b1, w2, b2, in_w, in_b, out_w, out_b,
              s=1.0):
    """Geometry + weight preprocessing.  `s` is the uint8 output scale:
    the device computes in x/s units (x pre-divided, w1 and the output
    projection compensated) and stores round(out/s)+128 as uint8."""
    import ml_dtypes
    F8NP = ml_dtypes.float8_e4m3
    points = np.asarray(points, np.float32)
    cavities = np.asarray(cavities, np.float32)
    d = np.sqrt(
        ((points[None, :, :] - cavities[:, None, :]) ** 2).sum(-1, dtype=np.float32)
    ).astype(np.float32)                                   # [K, P]
    mask = (d < RADIUS).astype(np.float32)                 # [K, P]
    counts = mask.sum(axis=1, dtype=np.float32)            # [K]
    inv = np.where(counts > 0, 1.0 / np.maximum(counts, 1.0), 0.0).astype(np.float32)

    maskA = np.zeros((P2, K), F8NP)
    maskA[:P] = mask.T.astype(F8NP)
    # [CHA, NA4*K]: maskA[p, c*K+k] = mask of point c*CHA+p, cavity k
    maskA = (maskA.reshape(NA4, CHA, K).transpose(1, 0, 2)
             .reshape(CHA, NA4 * K))

    nearest = np.argmin(d, axis=0)                         # [P]
    onehot = np.zeros((K, P2), F8NP)
    onehot[nearest, np.arange(P)] = F8NP(1.0)

    w1 = np.asarray(w1, np.float32)                        # [K, C2, C]
    w2 = np.asarray(w2, np.float32)                        # [K, C, C2]
    in_w = np.asarray(in_w, np.float32)
    in_b = np.asarray(in_b, np.float32)
    out_w = np.asarray(out_w, np.float32)
    out_b = np.asarray(out_b, np.float32)
    scale = np.float32(1.0 / np.sqrt(Dh))

    # w1c[c, k*C2+d] = w1[k,d,c] * inv[k]
    w1c = (w1 * (inv * np.float32(s))[:, None, None])\
        .transpose(2, 0, 1).reshape(C, K * C2)
    w1c = w1c.astype(F8NP)
    # w2c[dd, (k*2+j)*C + c] = w2[k, c, j*C+dd]
    w2c = (w2.transpose(0, 2, 1).reshape(K, 2, C, C)       # [k, j, dd, c]
           .transpose(2, 0, 1, 3).reshape(C, 2 * K * C)).astype(F8NP)
    wqkv = np.concatenate([in_w[0:C].T * scale, in_w[C:2 * C].T,
                           in_w[2 * C:3 * C].T], axis=1).astype(F8NP)
    fp = {
        "maskA": np.ascontiguousarray(maskA),
        "onehot": np.ascontiguousarray(onehot),
        "w1c": np.ascontiguousarray(w1c),
        "w2c": np.ascontiguousarray(w2c),
        "b1t": np.ascontiguousarray(
            b1.reshape(K, 2, C).transpose(2, 0, 1).reshape(C, 2 * K)
            .astype(np.float32)),
        "b2t": np.ascontiguousarray(np.asarray(b2, np.float32).T),   # [C, K]
        "wqkv": np.ascontiguousarray(wqkv),
        # wo_heads[d, h*C+e] = out_w[e, h*Dh+d]
        "wo": np.ascontiguousarray(
            (out_w / np.float32(s)).reshape(C, H, Dh)
            .transpose(2, 1, 0).reshape(Dh, H * C)),
        "qb": np.ascontiguousarray(np.repeat(
            (in_b[0:C] * scale).reshape(H, Dh).T, K, axis=1)),
        "kb": np.ascontiguousarray(np.repeat(
            in_b[C:2 * C].reshape(H, Dh).T, K, axis=1)),
        "vb": np.ascontiguousarray(np.tile(in_b[2 * C:3 * C], (K, 1))),
        "obr": np.ascontiguousarray(
            np.tile(out_b / np.float32(s) + np.float32(128.0), (K, 1))),
        "zc": np.zeros((C, 1), np.float32),
        "identT": np.eye(CHA, dtype=np.float16),
        "ident14": np.eye(K, dtype=np.float32),
    }
    return fp


_PROGRAM = None


def make_inputs(x, points, cavities, w1, b1, w2, b2, in_w, in_b, out_w,
                out_b):
    """Returns (in_maps, s): per-core input dicts + the int8 output scale."""
    x32 = np.asarray(x, np.float32)
    s = max(float(np.abs(x32).max()), 1e-20) * 1.01 / 127.0
    x16 = np.zeros((B, C, P2), np.float16)
    x16[:, :, :P] = x32 * np.float32(1.0 / s)
    fp = prep_host(points, cavities, w1, b1, w2, b2, in_w, in_b, out_w, out_b,
                   s=s)
    return [dict(fp, x=np.ascontiguousarray(x16[b])) for b in range(B)], s


def dequant(out, s):
    return (np.asarray(out, np.float32) - np.float32(128.0)) * np.float32(s)


def kernel(x, points, cavities, w1, b1, w2, b2, in_w, in_b, out_w, out_b):
    global _PROGRAM
    in_maps, s = make_inputs(x, points, cavities, w1, b1, w2, b2, in_w, in_b,
                             out_w, out_b)
    if _PROGRAM is None:
        _PROGRAM = build_program()
    res = run_bass_kernel_spmd(_PROGRAM, in_maps, list(range(B)))
    out = np.stack([np.asarray(res.results[b]["out"], np.float32)
                    for b in range(B)], axis=0)
    return (out - np.float32(128.0)) * np.float32(s)
